# revision 33
# baseline (speedup 1.0000x reference)
"""Causal self-attention (RoPE, 16 heads) on 8 Trainium2 NeuronCores — v3 fp8.

Sharding: core s -> (batch b = s//2, head-half g = s%2). Each core computes
qkv = x_b @ w_attn[:, heads g], RoPE, causal SDPA for its 8 heads, and a
partial y_local @ w_proj[rows g] -> [T, C]. Host sums the two partials per
batch (row-parallel Megatron unshard).

v3: fp8e4m3 DoubleRow matmuls with error compensation.
 - DoubleRow fp8 matmuls process two K-tiles per instruction at 0.5
   cycles/row (2x bf16 FLOPs, 4x when both slots carry fresh data).
 - qkv + proj gemms: 3-term compensated products over chunk pairs
   (a8@b8 + ar8@b8 + a8@br8 with (value, residual) fp8 pairs) = bf16-level
   accuracy at 0.75x the bf16 cycle cost. x/w/wp residuals from host.
 - scores: two DR matmuls per chunk: (k8,kr8) x q8-dup + (k8,kr8) x
   qr8-dup = exact (k8+kr8)(q8+qr8), so no score-path noise rides.
 - attention weights e8 = exp(s) in fp8 (noise ~1.4e-2, partially
   cancelled by the shared-denominator normalization).
 - causal mask: additive -57600 matmuls accumulated straight into the
   score PSUM group (identity x mask-pattern), no vector masking.
 - softmax denominator: ones-DR matmul accumulation over e8 pairs
   (column-sum broadcast to all partitions), recip on DVE.
 - RoPE: rotate on Pool (partition-shifted copies), combine muls split
   DVE/Pool, raw PSUM->SBUF copies on ACT.
"""

import sys

sys.path.insert(0, "/opt/trn_rl_repo")

import numpy as np

import concourse.bacc as bacc
import concourse.mybir as mybir
import concourse.tile as tile

P = 128
D = 128
F32 = mybir.dt.float32
BF16 = mybir.dt.bfloat16
FP8 = mybir.dt.float8e4
EXP = mybir.ActivationFunctionType.Exp
Dd = mybir.MatmulPerfMode.DoubleRow
MUL = mybir.AluOpType.mult
SUB = mybir.AluOpType.subtract

NUM_HEADS = 16
ROPE_THETA = 10000.0

# fp8 scales
SX = 16.0      # x8 = x * 16
SW = 512.0     # w8 = w * 512
SQK = 16.0     # q8/k8 post-rope
SV = 8.0       # v8
SY = 8.0       # y8
SWP = 512.0    # wp8
MBIG = 240.0   # mask magnitude (id2 * madd -> -57600 in score psum)


def build_nc(
    T=2048,
    CIN=2048,
    HL=8,
    COUT=2048,
    *,
    w_bufs=2,
    acc_bufs=2,
    st_bufs=3,
    e_bufs=3,
    qk_bufs=2,
    v_bufs=2,
    o_bufs=4,
    rope_bufs=2,
    tmp_bufs=2,
):
    CC = CIN // P        # contraction chunks (16)
    CP = CC // 2         # contraction chunk pairs (8)
    TB = T // 512        # 512-wide t blocks (4)
    TC = T // P          # 128-wide t chunks (16)
    NB = COUT // 512     # output col blocks (4)
    SCALE = 1.0 / float(np.sqrt(D))
    EXP_SCALE = SCALE / (SQK * SQK)     # score psum is q8*k8 = s*256
    V_SCALE = SV / (SX * SW)            # v psum is x8*w8 = v*8192
    O_SCALE = 1.0 / (SY * SWP)          # proj psum is y8*wp8 = out*4096

    nc = bacc.Bacc("TRN2", target_bir_lowering=False, debug=False)

    x2_d = nc.dram_tensor("x2", [2 * CIN, T], FP8, kind="ExternalInput").ap()
    w8_d = nc.dram_tensor("w8", [HL, 3, P, CC * D], FP8, kind="ExternalInput").ap()
    wr8_d = nc.dram_tensor("wr8", [HL, 3, P, CC * D], FP8, kind="ExternalInput").ap()
    wp8_d = nc.dram_tensor("wp8", [HL * D, COUT], FP8, kind="ExternalInput").ap()
    wpr8_d = nc.dram_tensor("wpr8", [HL * D, COUT], FP8, kind="ExternalInput").ap()
    cosT_d = nc.dram_tensor("cosT", [D, T], BF16, kind="ExternalInput").ap()
    sinT_d = nc.dram_tensor("sinT", [D, T], BF16, kind="ExternalInput").ap()
    madd_d = nc.dram_tensor("madd", [P, 2, 896], FP8, kind="ExternalInput").ap()
    id2_d = nc.dram_tensor("id2", [P, 2, P], FP8, kind="ExternalInput").ap()
    out_d = nc.dram_tensor("out", [T, COUT], BF16, kind="ExternalOutput").ap()

    with tile.TileContext(nc) as tc:
        with (
            tc.tile_pool(name="const", bufs=1) as cp,
            tc.tile_pool(name="xt", bufs=1) as xtp,
            tc.tile_pool(name="ropetab", bufs=1) as rtp,
            tc.tile_pool(name="maskp", bufs=1) as mp,
            tc.tile_pool(name="yt", bufs=1) as ytp,
            tc.tile_pool(name="wpp", bufs=1) as wpp,
            tc.tile_pool(name="w1", bufs=w_bufs) as wpool,
            tc.tile_pool(name="rope", bufs=rope_bufs) as rp,
            tc.tile_pool(name="qk", bufs=qk_bufs) as qkp,
            tc.tile_pool(name="vpool", bufs=v_bufs) as vp,
            tc.tile_pool(name="esb", bufs=e_bufs) as ep,
            tc.tile_pool(name="tmpp", bufs=tmp_bufs) as tp,
            tc.tile_pool(name="o", bufs=o_bufs) as op,
            tc.tile_pool(name="ps_acc", bufs=acc_bufs, space="PSUM") as accp,
            tc.tile_pool(name="ps_v", bufs=1, space="PSUM") as vpsp,
            tc.tile_pool(name="ps_st", bufs=st_bufs, space="PSUM") as stp,
            tc.tile_pool(name="ps_d", bufs=1, space="PSUM") as dpsp,
            tc.tile_pool(name="ps_y", bufs=1, space="PSUM") as ypsp,
        ):
            # constants
            ones8 = cp.tile([P, 2, P], FP8)
            nc.vector.memset(ones8[:], 1.0)

            # resident tensors (x8 and residual xr8 share one tile/stream)
            x2 = xtp.tile([P, 2, CC, T], FP8)
            x8 = x2[:, 0]
            xr8 = x2[:, 1]
            cosT = rtp.tile([D, T], BF16)
            sinT = rtp.tile([D, T], BF16)
            madd = mp.tile([P, 2, 896], FP8)
            id2 = mp.tile([P, 2, P], FP8)
            y8 = ytp.tile([P, HL, T], FP8)
            yr8 = ytp.tile([P, HL, T], FP8)
            wp8 = wpp.tile([P, HL, COUT], FP8)
            wpr8 = wpp.tile([P, HL, COUT], FP8)

            # --- startup DMAs (SP queue order matters) ---
            x2r = x2_d.rearrange("(s c p) t -> p s c t", p=P, s=2)
            w_tiles = {}

            def load_w(h, interleave=False):
                w8f = wpool.tile([P, 3 * CC * D], FP8, name="w8_h")
                wr8f = wpool.tile([P, 3 * CC * D], FP8, name="wr8_h")
                w8h = w8f.rearrange("p (j c d) -> p j c d", j=3, c=CC)
                wr8h = wr8f.rearrange("p (j c d) -> p j c d", j=3, c=CC)
                wv8 = w8_d[h].rearrange("j p k -> p j k")
                wvr8 = wr8_d[h].rearrange("j p k -> p j k")
                if interleave:
                    return (w8h, wr8h, w8f, wr8f, wv8, wvr8)
                nc.sync.dma_start(
                    w8f.rearrange("p (j k) -> p j k", j=3)[:], wv8
                )
                nc.sync.dma_start(
                    wr8f.rearrange("p (j k) -> p j k", j=3)[:], wvr8
                )
                w_tiles[h] = (w8h, wr8h)

            w8h0, wr8h0, w8f0, wr8f0, wv80, wvr80 = load_w(0, interleave=True)
            w_tiles[0] = (w8h0, wr8h0)
            KD = CC * D
            nc.sync.dma_start(x2[:, :, :, 0:128], x2r[:, :, :, 0:128])
            nc.sync.dma_start(w8f0[:, 0:KD], wv80[:, 0])
            nc.sync.dma_start(wr8f0[:, 0:KD], wvr80[:, 0])
            nc.sync.dma_start(x2[:, :, :, 128:256], x2r[:, :, :, 128:256])
            nc.sync.dma_start(w8f0[:, KD : 2 * KD], wv80[:, 1])
            nc.sync.dma_start(wr8f0[:, KD : 2 * KD], wvr80[:, 1])
            nc.sync.dma_start(cosT[:], cosT_d[:])
            nc.sync.dma_start(sinT[:], sinT_d[:])
            nc.sync.dma_start(w8f0[:, 2 * KD :], wv80[:, 2])
            nc.sync.dma_start(wr8f0[:, 2 * KD :], wvr80[:, 2])
            nc.sync.dma_start(x2[:, :, :, 256:512], x2r[:, :, :, 256:512])
            nc.sync.dma_start(madd[:], madd_d[:])
            nc.sync.dma_start(id2[:], id2_d[:])
            for tb in range(1, TB):
                s = slice(tb * 512, (tb + 1) * 512)
                nc.sync.dma_start(x2[:, :, :, s], x2r[:, :, :, s])

            qk_tiles = {}
            v_tiles = {}

            def rope_combine(j, acc, h, b, lo, hi):
                """acc (psum, q*8192) -> q8 / (k8, kr8) fp8 at scale 16."""
                w_ = hi - lo
                hs = slice(b * 512 + lo, b * 512 + hi)
                q8t, k82 = qk_tiles[h]
                raw = rp.tile([P, 512], BF16, name="raw")
                # b==0 pieces run beside a 16-chunk att block: ACT is
                # saturated with exps there, so spill the raw copy to DVE
                if b == 0:
                    nc.vector.tensor_copy(raw[:, 0:w_], acc[:, 0:w_])
                else:
                    nc.scalar.copy(raw[:, 0:w_], acc[:, 0:w_])
                rot = rp.tile([P, 512], BF16, name="rot")
                nc.vector.tensor_scalar_mul(
                    rot[0:64, 0:w_], raw[64:128, 0:w_], -1.0
                )
                nc.vector.tensor_copy(rot[64:128, 0:w_], raw[0:64, 0:w_])
                qc = rp.tile([P, 512], BF16, name="qc")
                # cos/sin tables pre-scaled by SQK/(SX*SW) on host
                nc.vector.tensor_mul(qc[:, 0:w_], raw[:, 0:w_], cosT[:, hs])
                qs = rp.tile([P, 512], BF16, name="qs")
                nc.gpsimd.tensor_mul(qs[:, 0:w_], rot[:, 0:w_], sinT[:, hs])
                dst = q8t if j == 0 else k82
                kbf = rp.tile([P, 512], BF16, name="kbf")
                nc.vector.tensor_add(kbf[:, 0:w_], qc[:, 0:w_], qs[:, 0:w_])
                nc.gpsimd.tensor_copy(dst[:, 0, hs], kbf[:, 0:w_])
                nc.vector.tensor_tensor(
                    dst[:, 1, hs], kbf[:, 0:w_], dst[:, 0, hs], SUB
                )

            def qkv_piece(h, b):
                """q8/k82 for t-block b (with RoPE) + v8/vr8 t-chunks 4b..4b+3."""
                w8h, wr8h = w_tiles[h]
                if b == 0:
                    q8t = qkp.tile([P, 2, T], FP8, name="q8t")
                    k82 = qkp.tile([P, 2, T], FP8, name="k82")
                    v8 = vp.tile([P, TC, D], FP8, name="v8")
                    vr8 = vp.tile([P, TC, D], FP8, name="vr8")
                    qk_tiles[h] = (q8t, k82)
                    v_tiles[h] = (v8, vr8)
                v8, vr8 = v_tiles[h]
                halves = (
                    [(0, 128), (128, 256), (256, 512)]
                    if (h == 0 and b == 0)
                    else [(0, 512)]
                )
                for lo, hi in halves:
                    w_ = hi - lo
                    hs = slice(b * 512 + lo, b * 512 + hi)
                    for j in range(2):  # q, k gemms: 3-term over chunk pairs
                        acc = accp.tile([P, 512], F32, name="acc")
                        for cpi in range(CP):
                            c = 2 * cpi
                            cs = slice(c, c + 2)
                            nc.tensor.matmul(
                                acc[:, 0:w_],
                                w8h[:, j, cs],
                                x8[:, cs, hs],
                                start=(cpi == 0),
                                stop=False,
                                perf_mode=Dd,
                            )
                            nc.tensor.matmul(
                                acc[:, 0:w_],
                                wr8h[:, j, cs],
                                x8[:, cs, hs],
                                start=False,
                                stop=False,
                                perf_mode=Dd,
                            )
                            nc.tensor.matmul(
                                acc[:, 0:w_],
                                w8h[:, j, cs],
                                xr8[:, cs, hs],
                                start=False,
                                stop=(cpi == CP - 1),
                                perf_mode=Dd,
                            )
                        rope_combine(j, acc, h, b, lo, hi)
                    # V in [t, d] layout: x-stationary 3-term
                    vps = vpsp.tile([P, 4, P], F32, name="vps")
                    t4s = range(lo // P, hi // P)
                    for t4 in t4s:
                        tt = 4 * b + t4
                        ts_ = slice(tt * P, (tt + 1) * P)
                        for cpi in range(CP):
                            cs = slice(2 * cpi, 2 * cpi + 2)
                            nc.tensor.matmul(
                                vps[:, t4],
                                x8[:, cs, ts_],
                                w8h[:, 2, cs],
                                start=(cpi == 0),
                                stop=False,
                                perf_mode=Dd,
                            )
                            nc.tensor.matmul(
                                vps[:, t4],
                                xr8[:, cs, ts_],
                                w8h[:, 2, cs],
                                start=False,
                                stop=False,
                                perf_mode=Dd,
                            )
                            nc.tensor.matmul(
                                vps[:, t4],
                                x8[:, cs, ts_],
                                wr8h[:, 2, cs],
                                start=False,
                                stop=(cpi == CP - 1),
                                perf_mode=Dd,
                            )
                    vsl = slice(4 * b + t4s.start, 4 * b + t4s.stop)
                    if b == 0:
                        nc.vector.tensor_scalar_mul(
                            v8[:, vsl, :], vps[:, t4s.start : t4s.stop], V_SCALE
                        )
                    else:
                        nc.scalar.mul(
                            v8[:, vsl, :], vps[:, t4s.start : t4s.stop], V_SCALE
                        )
                    nc.vector.scalar_tensor_tensor(
                        vr8[:, vsl, :],
                        vps[:, t4s.start : t4s.stop],
                        V_SCALE,
                        v8[:, vsl, :],
                        MUL,
                        SUB,
                    )

            att_state = {}

            def att_block(h, b, filler=None, c_lo=0, c_hi=None):
                """Causal attention for head h, q block b -> y8/yr8[:, h, block].

                Steps [c_lo, c_hi) of the chunk loop; a big block can be
                split across two qkv pieces to level the ACT exp load.
                """
                q8t, k82 = qk_tiles[h]
                v8, vr8 = v_tiles[h]
                nch = 4 * (b + 1)
                npair = nch // 2
                bs = slice(b * 512, (b + 1) * 512)
                if c_hi is None:
                    c_hi = nch + 3
                if c_lo == 0:
                    yps = ypsp.tile([P, 512], F32, name="yps")
                    dps = dpsp.tile([P, 512], F32, name="dps")
                    es = {}
                    att_state[(h, b)] = (yps, dps, es)
                else:
                    yps, dps, es = att_state[(h, b)]
                qlo = {nch - 2: 256, nch - 1: 256}
                # software-pipelined: y(pair p) trails scores by 2 pairs so
                # the ACT exp chain is never on the PE critical path
                for c in range(c_lo, c_hi):
                    if c < nch:
                        lo = qlo.get(c, 0)
                        w_ = 512 - lo
                        qs_ = slice(b * 512 + lo, (b + 1) * 512)
                        st = stp.tile([P, 512], F32, name="st")
                        j = c - (nch - 4)
                        ks_ = k82[:, :, c * P : (c + 1) * P]
                        nc.tensor.matmul(
                            st[:, 0:w_],
                            ks_,
                            q8t[:, 0:1, qs_].broadcast_to((P, 2, w_)),
                            start=True,
                            stop=False,
                            perf_mode=Dd,
                        )
                        nc.tensor.matmul(
                            st[:, 0:w_],
                            ks_,
                            q8t[:, 1:2, qs_].broadcast_to((P, 2, w_)),
                            start=False,
                            stop=(j < 0),
                            perf_mode=Dd,
                        )
                        if j >= 0:
                            # additive causal mask into the same psum group
                            wj = 128 * (j + 1) - lo
                            ms = 384 - 128 * j + lo
                            nc.tensor.matmul(
                                st[:, 0:wj],
                                id2[:],
                                madd[:, :, ms : ms + wj],
                                start=False,
                                stop=True,
                                perf_mode=Dd,
                            )
                        if c % 2 == 0:
                            e2 = ep.tile([P, 2, 512], FP8, name="e2")
                            es[c // 2] = e2
                        e2 = es[c // 2]
                        nc.scalar.activation(
                            e2[:, c % 2, 0:w_], st[:, 0:w_], EXP, scale=EXP_SCALE
                        )
                    if c >= 4 and c % 2 == 0:
                        p = c // 2 - 2
                        lo = qlo.get(2 * p, 0)
                        w_ = 512 - lo
                        e2p = es.pop(p)
                        vs = slice(2 * p, 2 * p + 2)
                        nc.tensor.matmul(
                            yps[:, lo:512],
                            v8[:, vs, :],
                            e2p[:, :, 0:w_],
                            start=(p == 0),
                            stop=False,
                            perf_mode=Dd,
                        )
                        nc.tensor.matmul(
                            yps[:, lo:512],
                            vr8[:, vs, :],
                            e2p[:, :, 0:w_],
                            start=False,
                            stop=(p == npair - 1),
                            perf_mode=Dd,
                        )
                        nc.tensor.matmul(
                            dps[:, lo:512],
                            ones8[:],
                            e2p[:, :, 0:w_],
                            start=(p == 0),
                            stop=(p == npair - 1),
                            perf_mode=Dd,
                        )
                        if filler:
                            filler.pop(0)()
                if c_hi < nch + 3:
                    return
                att_state.pop((h, b))
                recip = tp.tile([P, 512], BF16, name="recip")
                with nc.allow_low_precision(reason="bf16 softmax recip"):
                    nc.vector.reciprocal(recip[:], dps[:])
                tmp = tp.tile([P, 512], BF16, name="tmp")
                nc.vector.tensor_mul(tmp[:], yps[:], recip[:])
                nc.gpsimd.tensor_copy(y8[:, h, bs], tmp[:])
                nc.gpsimd.tensor_tensor(yr8[:, h, bs], tmp[:], y8[:, h, bs], SUB)

            def proj_tile(tt, nb):
                """One out tile: out[tt, nb] = sum_h y[:,h,tt].T @ wp (3-term)."""
                g = tt * NB + nb
                pool = vpsp if g % 3 == 2 else accp
                name = "vps" if g % 3 == 2 else "acc"
                ps3 = pool.tile([P, 512], F32, name=name)
                ts_ = slice(tt * P, (tt + 1) * P)
                ns = slice(nb * 512, (nb + 1) * 512)
                for hp in range(HL // 2):
                    hsl = slice(2 * hp, 2 * hp + 2)
                    nc.tensor.matmul(
                        ps3[:],
                        y8[:, hsl, ts_],
                        wp8[:, hsl, ns],
                        start=(hp == 0),
                        stop=False,
                        perf_mode=Dd,
                    )
                    nc.tensor.matmul(
                        ps3[:],
                        yr8[:, hsl, ts_],
                        wp8[:, hsl, ns],
                        start=False,
                        stop=False,
                        perf_mode=Dd,
                    )
                    nc.tensor.matmul(
                        ps3[:],
                        y8[:, hsl, ts_],
                        wpr8[:, hsl, ns],
                        start=False,
                        stop=(hp == HL // 2 - 1),
                        perf_mode=Dd,
                    )
                o_sb = op.tile([P, 512], BF16, name="o_sb")
                if g % 2 == 0:
                    nc.scalar.mul(o_sb[:], ps3[:], O_SCALE)
                else:
                    nc.vector.tensor_scalar_mul(o_sb[:], ps3[:], O_SCALE)
                nc.sync.dma_start(out_d[ts_, ns], o_sb[:])

            def proj_thunks(b):
                return [
                    (lambda tt=4 * b + t4, nb=nb: proj_tile(tt, nb))
                    for t4 in range(4)
                    for nb in range(NB)
                ]

            # --- fused pipeline: attention trails qkv by one piece; the
            # 16-chunk b=3 block is split across two pieces to level the
            # ACT exp load ---
            for h in range(HL):
                if h + 1 < HL:
                    load_w(h + 1)  # prefetch next head's weights
                if h == 2:
                    nc.sync.dma_start(
                        wp8[:], wp8_d.rearrange("(h p) n -> p h n", p=P)
                    )
                    nc.sync.dma_start(
                        wpr8[:], wpr8_d.rearrange("(h p) n -> p h n", p=P)
                    )
                for b in range(TB):
                    qkv_piece(h, b)
                    if h == 0:
                        if b >= 1:
                            att_block(0, b - 1)
                    elif b == 0:
                        att_block(h - 1, 3)
                    else:
                        att_block(h, b - 1)
            avail = []
            for b in range(TB - 1):
                avail.extend(proj_thunks(b))
            att_block(HL - 1, TB - 1, filler=avail)
            avail.extend(proj_thunks(TB - 1))
            for t in avail:
                t()

    nc.compile()
    return nc


def _rope_tables_T(T, head_dim):
    half = head_dim // 2
    inv_freq = 1.0 / (ROPE_THETA ** (np.arange(0, half, dtype=np.float64) / half))
    ang = np.arange(T, dtype=np.float64)[:, None] * inv_freq[None, :]  # [T, half]
    cos = np.concatenate([np.cos(ang), np.cos(ang)], axis=-1)  # [T, D]
    sin = np.concatenate([np.sin(ang), np.sin(ang)], axis=-1)
    return (
        np.ascontiguousarray(cos.T.astype(np.float32)),
        np.ascontiguousarray(sin.T.astype(np.float32)),
    )


_NC_CACHE = {}


def _get_nc(T, CIN, HL, COUT):
    key = (T, CIN, HL, COUT)
    if key not in _NC_CACHE:
        _NC_CACHE[key] = build_nc(T, CIN, HL, COUT)
    return _NC_CACHE[key]


def make_in_maps(x, w_attn, w_proj):
    import ml_dtypes

    f8 = ml_dtypes.float8_e4m3
    bf16 = ml_dtypes.bfloat16

    def q8pair(a, s):
        v8 = (np.asarray(a, np.float32) * s).astype(f8)
        r8 = (np.asarray(a, np.float32) * s - v8.astype(np.float32)).astype(f8)
        return v8, r8

    x = np.asarray(x)
    w_attn = np.asarray(w_attn)
    w_proj = np.asarray(w_proj)
    B, T, C = x.shape
    HL = NUM_HEADS // 2  # 8 heads per core
    CL = HL * D  # 1024
    CC = C // P

    cosT, sinT = _rope_tables_T(T, D)
    # fold SQK/(SX*SW) into the tables: psum is q*8192, out target q*16
    tab_scale = SQK / (SX * SW)
    cosT = (cosT * tab_scale).astype(bf16)
    sinT = (sinT * tab_scale).astype(bf16)

    # additive causal mask pattern: madd[p, 0, u] = -MBIG if u < p+384
    u = np.arange(896)[None, :]
    pp = np.arange(P)[:, None]
    madd = np.zeros((P, 2, 896), np.float32)
    madd[:, 0, :] = np.where(u < pp + 384, -MBIG, 0.0)
    madd = madd.astype(f8)
    id2 = np.zeros((P, 2, P), np.float32)
    id2[:, 0, :] = np.eye(P) * MBIG
    id2 = id2.astype(f8)

    wp_shards = []
    w_shards = []
    for g in range(2):
        qkv_cols = [
            w_attn[:, g * CL : (g + 1) * CL],
            w_attn[:, C + g * CL : C + (g + 1) * CL],
            w_attn[:, 2 * C + g * CL : 2 * C + (g + 1) * CL],
        ]
        w_shard = np.empty((HL, 3, P, CC * D), dtype=np.float32)
        for j, wj in enumerate(qkv_cols):
            s = wj.reshape(CC, P, HL, D).transpose(2, 1, 0, 3)  # [HL, P, CC, D]
            w_shard[:, j] = s.reshape(HL, P, CC * D)
        w_shards.append(q8pair(w_shard, SW))
        wp_shards.append(
            q8pair(np.ascontiguousarray(w_proj[g * CL : (g + 1) * CL, :]), SWP)
        )

    in_maps = []
    for s in range(8):
        b, g = s // 2, s % 2
        xT = np.ascontiguousarray(x[b].T)
        x8, xr8 = q8pair(xT, SX)
        x2 = np.ascontiguousarray(np.stack([x8, xr8], axis=0)).reshape(2 * C, T)
        w8, wr8 = w_shards[g]
        wp8, wpr8 = wp_shards[g]
        in_maps.append(
            {
                "x2": x2,
                "w8": w8,
                "wr8": wr8,
                "wp8": wp8,
                "wpr8": wpr8,
                "cosT": cosT,
                "sinT": sinT,
                "madd": madd,
                "id2": id2,
            }
        )
    return in_maps


def combine(results, x_shape):
    B, T, C = x_shape
    out = np.empty((B, T, C), dtype=np.float32)
    for b in range(B):
        out[b] = results[2 * b]["out"].astype(np.float32) + results[
            2 * b + 1
        ]["out"].astype(np.float32)
    return out


def kernel(x, w_attn, w_proj):
    from concourse.bass_utils import run_bass_kernel_spmd

    x = np.asarray(x)
    B, T, C = x.shape  # 4, 2048, 2048
    HL = NUM_HEADS // 2

    nc = _get_nc(T, C, HL, C)
    in_maps = make_in_maps(x, w_attn, w_proj)
    res = run_bass_kernel_spmd(nc, in_maps, list(range(8)))
    return combine(res.results, (B, T, C))


# revision 34
# speedup vs baseline: 1.0018x; 1.0018x over previous
"""Causal self-attention (RoPE, 16 heads) on 8 Trainium2 NeuronCores — v3 fp8.

Sharding: core s -> (batch b = s//2, head-half g = s%2). Each core computes
qkv = x_b @ w_attn[:, heads g], RoPE, causal SDPA for its 8 heads, and a
partial y_local @ w_proj[rows g] -> [T, C]. Host sums the two partials per
batch (row-parallel Megatron unshard).

v3: fp8e4m3 DoubleRow matmuls with error compensation.
 - DoubleRow fp8 matmuls process two K-tiles per instruction at 0.5
   cycles/row (2x bf16 FLOPs, 4x when both slots carry fresh data).
 - qkv + proj gemms: 3-term compensated products over chunk pairs
   (a8@b8 + ar8@b8 + a8@br8 with (value, residual) fp8 pairs) = bf16-level
   accuracy at 0.75x the bf16 cycle cost. x/w/wp residuals from host.
 - scores: two DR matmuls per chunk: (k8,kr8) x q8-dup + (k8,kr8) x
   qr8-dup = exact (k8+kr8)(q8+qr8), so no score-path noise rides.
 - attention weights e8 = exp(s) in fp8 (noise ~1.4e-2, partially
   cancelled by the shared-denominator normalization).
 - causal mask: additive -57600 matmuls accumulated straight into the
   score PSUM group (identity x mask-pattern), no vector masking.
 - softmax denominator: ones-DR matmul accumulation over e8 pairs
   (column-sum broadcast to all partitions), recip on DVE.
 - RoPE: rotate on Pool (partition-shifted copies), combine muls split
   DVE/Pool, raw PSUM->SBUF copies on ACT.
"""

import sys

sys.path.insert(0, "/opt/trn_rl_repo")

import numpy as np

import concourse.bacc as bacc
import concourse.mybir as mybir
import concourse.tile as tile

P = 128
D = 128
F32 = mybir.dt.float32
BF16 = mybir.dt.bfloat16
FP8 = mybir.dt.float8e4
EXP = mybir.ActivationFunctionType.Exp
Dd = mybir.MatmulPerfMode.DoubleRow
MUL = mybir.AluOpType.mult
SUB = mybir.AluOpType.subtract

NUM_HEADS = 16
ROPE_THETA = 10000.0

# fp8 scales
SX = 16.0      # x8 = x * 16
SW = 512.0     # w8 = w * 512
SQK = 16.0     # q8/k8 post-rope
SV = 8.0       # v8
SY = 8.0       # y8
SWP = 512.0    # wp8
MBIG = 240.0   # mask magnitude (id2 * madd -> -57600 in score psum)


def build_nc(
    T=2048,
    CIN=2048,
    HL=8,
    COUT=2048,
    *,
    w_bufs=2,
    acc_bufs=2,
    st_bufs=3,
    e_bufs=3,
    qk_bufs=2,
    v_bufs=2,
    o_bufs=4,
    rope_bufs=2,
    tmp_bufs=2,
):
    CC = CIN // P        # contraction chunks (16)
    CP = CC // 2         # contraction chunk pairs (8)
    TB = T // 512        # 512-wide t blocks (4)
    TC = T // P          # 128-wide t chunks (16)
    NB = COUT // 512     # output col blocks (4)
    SCALE = 1.0 / float(np.sqrt(D))
    EXP_SCALE = SCALE / (SQK * SQK)     # score psum is q8*k8 = s*256
    V_SCALE = SV / (SX * SW)            # v psum is x8*w8 = v*8192
    O_SCALE = 1.0 / (SY * SWP)          # proj psum is y8*wp8 = out*4096

    nc = bacc.Bacc("TRN2", target_bir_lowering=False, debug=False)

    x2_d = nc.dram_tensor("x2", [2 * CIN, T], FP8, kind="ExternalInput").ap()
    w8_d = nc.dram_tensor("w8", [HL, 3, P, CC * D], FP8, kind="ExternalInput").ap()
    wr8_d = nc.dram_tensor("wr8", [HL, 3, P, CC * D], FP8, kind="ExternalInput").ap()
    wp8_d = nc.dram_tensor("wp8", [HL * D, COUT], FP8, kind="ExternalInput").ap()
    wpr8_d = nc.dram_tensor("wpr8", [HL * D, COUT], FP8, kind="ExternalInput").ap()
    cosT_d = nc.dram_tensor("cosT", [D, T], BF16, kind="ExternalInput").ap()
    sinT_d = nc.dram_tensor("sinT", [D, T], BF16, kind="ExternalInput").ap()
    madd_d = nc.dram_tensor("madd", [P, 2, 896], FP8, kind="ExternalInput").ap()
    id2_d = nc.dram_tensor("id2", [P, 2, P], FP8, kind="ExternalInput").ap()
    out_d = nc.dram_tensor("out", [T, COUT], BF16, kind="ExternalOutput").ap()

    with tile.TileContext(nc) as tc:
        with (
            tc.tile_pool(name="const", bufs=1) as cp,
            tc.tile_pool(name="xt", bufs=1) as xtp,
            tc.tile_pool(name="ropetab", bufs=1) as rtp,
            tc.tile_pool(name="maskp", bufs=1) as mp,
            tc.tile_pool(name="yt", bufs=1) as ytp,
            tc.tile_pool(name="wpp", bufs=1) as wpp,
            tc.tile_pool(name="w1", bufs=w_bufs) as wpool,
            tc.tile_pool(name="rope", bufs=rope_bufs) as rp,
            tc.tile_pool(name="qk", bufs=qk_bufs) as qkp,
            tc.tile_pool(name="vpool", bufs=v_bufs) as vp,
            tc.tile_pool(name="esb", bufs=e_bufs) as ep,
            tc.tile_pool(name="tmpp", bufs=tmp_bufs) as tp,
            tc.tile_pool(name="o", bufs=o_bufs) as op,
            tc.tile_pool(name="ps_acc", bufs=acc_bufs, space="PSUM") as accp,
            tc.tile_pool(name="ps_v", bufs=1, space="PSUM") as vpsp,
            tc.tile_pool(name="ps_st", bufs=st_bufs, space="PSUM") as stp,
            tc.tile_pool(name="ps_d", bufs=1, space="PSUM") as dpsp,
            tc.tile_pool(name="ps_y", bufs=1, space="PSUM") as ypsp,
        ):
            # constants
            ones8 = cp.tile([P, 2, P], FP8)
            nc.vector.memset(ones8[:], 1.0)

            # resident tensors (x8 and residual xr8 share one tile/stream)
            x2 = xtp.tile([P, 2, CC, T], FP8)
            x8 = x2[:, 0]
            xr8 = x2[:, 1]
            cosT = rtp.tile([D, T], BF16)
            sinT = rtp.tile([D, T], BF16)
            madd = mp.tile([P, 2, 896], FP8)
            id2 = mp.tile([P, 2, P], FP8)
            y8 = ytp.tile([P, HL, T], FP8)
            yr8 = ytp.tile([P, HL, T], FP8)
            wp8 = wpp.tile([P, HL, COUT], FP8)
            wpr8 = wpp.tile([P, HL, COUT], FP8)

            # --- startup DMAs (SP queue order matters) ---
            x2r = x2_d.rearrange("(s c p) t -> p s c t", p=P, s=2)
            w_tiles = {}

            def load_w(h, interleave=False):
                w8f = wpool.tile([P, 3 * CC * D], FP8, name="w8_h")
                wr8f = wpool.tile([P, 3 * CC * D], FP8, name="wr8_h")
                w8h = w8f.rearrange("p (j c d) -> p j c d", j=3, c=CC)
                wr8h = wr8f.rearrange("p (j c d) -> p j c d", j=3, c=CC)
                wv8 = w8_d[h].rearrange("j p k -> p j k")
                wvr8 = wr8_d[h].rearrange("j p k -> p j k")
                if interleave:
                    return (w8h, wr8h, w8f, wr8f, wv8, wvr8)
                nc.sync.dma_start(
                    w8f.rearrange("p (j k) -> p j k", j=3)[:], wv8
                )
                nc.sync.dma_start(
                    wr8f.rearrange("p (j k) -> p j k", j=3)[:], wvr8
                )
                w_tiles[h] = (w8h, wr8h)

            w8h0, wr8h0, w8f0, wr8f0, wv80, wvr80 = load_w(0, interleave=True)
            w_tiles[0] = (w8h0, wr8h0)
            KD = CC * D
            nc.sync.dma_start(x2[:, :, :, 0:128], x2r[:, :, :, 0:128])
            nc.sync.dma_start(w8f0[:, 0:KD], wv80[:, 0])
            nc.sync.dma_start(wr8f0[:, 0:KD], wvr80[:, 0])
            nc.sync.dma_start(x2[:, :, :, 128:256], x2r[:, :, :, 128:256])
            nc.sync.dma_start(w8f0[:, KD : 2 * KD], wv80[:, 1])
            nc.sync.dma_start(wr8f0[:, KD : 2 * KD], wvr80[:, 1])
            nc.sync.dma_start(cosT[:], cosT_d[:])
            nc.sync.dma_start(sinT[:], sinT_d[:])
            nc.sync.dma_start(w8f0[:, 2 * KD :], wv80[:, 2])
            nc.sync.dma_start(wr8f0[:, 2 * KD :], wvr80[:, 2])
            nc.sync.dma_start(x2[:, :, :, 256:512], x2r[:, :, :, 256:512])
            nc.sync.dma_start(madd[:], madd_d[:])
            nc.sync.dma_start(id2[:], id2_d[:])
            for tb in range(1, TB):
                s = slice(tb * 512, (tb + 1) * 512)
                nc.sync.dma_start(x2[:, :, :, s], x2r[:, :, :, s])

            qk_tiles = {}
            v_tiles = {}

            def rope_combine(j, acc, h, b, lo, hi):
                """acc (psum, q*8192) -> q8 / (k8, kr8) fp8 at scale 16."""
                w_ = hi - lo
                hs = slice(b * 512 + lo, b * 512 + hi)
                q8t, k82 = qk_tiles[h]
                raw = rp.tile([P, 512], BF16, name="raw")
                # b==0 pieces run beside a 16-chunk att block: ACT is
                # saturated with exps there, so spill the raw copy to DVE
                if b == 0:
                    nc.vector.tensor_copy(raw[:, 0:w_], acc[:, 0:w_])
                else:
                    nc.scalar.copy(raw[:, 0:w_], acc[:, 0:w_])
                rot = rp.tile([P, 512], BF16, name="rot")
                nc.vector.tensor_scalar_mul(
                    rot[0:64, 0:w_], raw[64:128, 0:w_], -1.0
                )
                nc.vector.tensor_copy(rot[64:128, 0:w_], raw[0:64, 0:w_])
                qc = rp.tile([P, 512], BF16, name="qc")
                # cos/sin tables pre-scaled by SQK/(SX*SW) on host
                nc.vector.tensor_mul(qc[:, 0:w_], raw[:, 0:w_], cosT[:, hs])
                qs = rp.tile([P, 512], BF16, name="qs")
                nc.gpsimd.tensor_mul(qs[:, 0:w_], rot[:, 0:w_], sinT[:, hs])
                dst = q8t if j == 0 else k82
                kbf = rp.tile([P, 512], BF16, name="kbf")
                nc.vector.tensor_add(kbf[:, 0:w_], qc[:, 0:w_], qs[:, 0:w_])
                nc.gpsimd.tensor_copy(dst[:, 0, hs], kbf[:, 0:w_])
                nc.vector.tensor_tensor(
                    dst[:, 1, hs], kbf[:, 0:w_], dst[:, 0, hs], SUB
                )

            def qkv_piece(h, b):
                """q8/k82 for t-block b (with RoPE) + v8/vr8 t-chunks 4b..4b+3."""
                w8h, wr8h = w_tiles[h]
                if b == 0:
                    q8t = qkp.tile([P, 2, T], FP8, name="q8t")
                    k82 = qkp.tile([P, 2, T], FP8, name="k82")
                    v8 = vp.tile([P, TC, D], FP8, name="v8")
                    vr8 = vp.tile([P, TC, D], FP8, name="vr8")
                    qk_tiles[h] = (q8t, k82)
                    v_tiles[h] = (v8, vr8)
                v8, vr8 = v_tiles[h]
                halves = (
                    [(0, 128), (128, 256), (256, 512)]
                    if (h == 0 and b == 0)
                    else [(0, 512)]
                )
                for lo, hi in halves:
                    w_ = hi - lo
                    hs = slice(b * 512 + lo, b * 512 + hi)
                    for j in range(2):  # q, k gemms: 3-term over chunk pairs
                        acc = accp.tile([P, 512], F32, name="acc")
                        for cpi in range(CP):
                            c = 2 * cpi
                            cs = slice(c, c + 2)
                            nc.tensor.matmul(
                                acc[:, 0:w_],
                                w8h[:, j, cs],
                                x8[:, cs, hs],
                                start=(cpi == 0),
                                stop=False,
                                perf_mode=Dd,
                            )
                            nc.tensor.matmul(
                                acc[:, 0:w_],
                                wr8h[:, j, cs],
                                x8[:, cs, hs],
                                start=False,
                                stop=False,
                                perf_mode=Dd,
                            )
                            nc.tensor.matmul(
                                acc[:, 0:w_],
                                w8h[:, j, cs],
                                xr8[:, cs, hs],
                                start=False,
                                stop=(cpi == CP - 1),
                                perf_mode=Dd,
                            )
                        rope_combine(j, acc, h, b, lo, hi)
                    # V in [t, d] layout: x-stationary 3-term
                    vps = vpsp.tile([P, 4, P], F32, name="vps")
                    t4s = range(lo // P, hi // P)
                    for t4 in t4s:
                        tt = 4 * b + t4
                        ts_ = slice(tt * P, (tt + 1) * P)
                        for cpi in range(CP):
                            cs = slice(2 * cpi, 2 * cpi + 2)
                            nc.tensor.matmul(
                                vps[:, t4],
                                x8[:, cs, ts_],
                                w8h[:, 2, cs],
                                start=(cpi == 0),
                                stop=False,
                                perf_mode=Dd,
                            )
                            nc.tensor.matmul(
                                vps[:, t4],
                                xr8[:, cs, ts_],
                                w8h[:, 2, cs],
                                start=False,
                                stop=False,
                                perf_mode=Dd,
                            )
                            nc.tensor.matmul(
                                vps[:, t4],
                                x8[:, cs, ts_],
                                wr8h[:, 2, cs],
                                start=False,
                                stop=(cpi == CP - 1),
                                perf_mode=Dd,
                            )
                    vsl = slice(4 * b + t4s.start, 4 * b + t4s.stop)
                    if b == 0:
                        nc.vector.tensor_scalar_mul(
                            v8[:, vsl, :], vps[:, t4s.start : t4s.stop], V_SCALE
                        )
                    else:
                        nc.scalar.mul(
                            v8[:, vsl, :], vps[:, t4s.start : t4s.stop], V_SCALE
                        )
                    nc.vector.scalar_tensor_tensor(
                        vr8[:, vsl, :],
                        vps[:, t4s.start : t4s.stop],
                        V_SCALE,
                        v8[:, vsl, :],
                        MUL,
                        SUB,
                    )

            att_state = {}

            def att_block(h, b, filler=None, c_lo=0, c_hi=None):
                """Causal attention for head h, q block b -> y8/yr8[:, h, block].

                Steps [c_lo, c_hi) of the chunk loop; a big block can be
                split across two qkv pieces to level the ACT exp load.
                """
                q8t, k82 = qk_tiles[h]
                v8, vr8 = v_tiles[h]
                nch = 4 * (b + 1)
                npair = nch // 2
                bs = slice(b * 512, (b + 1) * 512)
                if c_hi is None:
                    c_hi = nch + 3
                if c_lo == 0:
                    yps = ypsp.tile([P, 512], F32, name="yps")
                    dps = dpsp.tile([P, 512], F32, name="dps")
                    es = {}
                    att_state[(h, b)] = (yps, dps, es)
                else:
                    yps, dps, es = att_state[(h, b)]
                qlo = {nch - 2: 256, nch - 1: 256}
                # software-pipelined: y(pair p) trails scores by 2 pairs so
                # the ACT exp chain is never on the PE critical path
                for c in range(c_lo, c_hi):
                    if c < nch:
                        lo = qlo.get(c, 0)
                        w_ = 512 - lo
                        st = stp.tile([P, 512], F32, name="st")
                        j = c - (nch - 4)
                        ks_ = k82[:, :, c * P : (c + 1) * P]
                        if c == nch - 1:
                            # last diagonal chunk: cols [0:128) of the lo=256
                            # window are fully masked -- fill them with the
                            # constant mask-add alone and only compute scores
                            # for the top 128 columns
                            nc.tensor.matmul(
                                st[:, 0:128],
                                id2[:],
                                madd[:, :, 256:384],
                                start=True,
                                stop=True,
                                perf_mode=Dd,
                            )
                            slo = 384
                        else:
                            slo = lo
                        sw = 512 - slo
                        soff = slo - lo
                        qs_ = slice(b * 512 + slo, (b + 1) * 512)
                        nc.tensor.matmul(
                            st[:, soff : soff + sw],
                            ks_,
                            q8t[:, 0:1, qs_].broadcast_to((P, 2, sw)),
                            start=True,
                            stop=False,
                            perf_mode=Dd,
                        )
                        nc.tensor.matmul(
                            st[:, soff : soff + sw],
                            ks_,
                            q8t[:, 1:2, qs_].broadcast_to((P, 2, sw)),
                            start=False,
                            stop=(j < 0),
                            perf_mode=Dd,
                        )
                        if j >= 0:
                            # additive causal mask into the same psum group
                            wj = 128 * (j + 1) - slo
                            ms = 384 - 128 * j + slo
                            nc.tensor.matmul(
                                st[:, soff : soff + wj],
                                id2[:],
                                madd[:, :, ms : ms + wj],
                                start=False,
                                stop=True,
                                perf_mode=Dd,
                            )
                        if c % 2 == 0:
                            e2 = ep.tile([P, 2, 512], FP8, name="e2")
                            es[c // 2] = e2
                        e2 = es[c // 2]
                        nc.scalar.activation(
                            e2[:, c % 2, 0:w_], st[:, 0:w_], EXP, scale=EXP_SCALE
                        )
                    if c >= 4 and c % 2 == 0:
                        p = c // 2 - 2
                        lo = qlo.get(2 * p, 0)
                        w_ = 512 - lo
                        e2p = es.pop(p)
                        vs = slice(2 * p, 2 * p + 2)
                        nc.tensor.matmul(
                            yps[:, lo:512],
                            v8[:, vs, :],
                            e2p[:, :, 0:w_],
                            start=(p == 0),
                            stop=False,
                            perf_mode=Dd,
                        )
                        nc.tensor.matmul(
                            yps[:, lo:512],
                            vr8[:, vs, :],
                            e2p[:, :, 0:w_],
                            start=False,
                            stop=(p == npair - 1),
                            perf_mode=Dd,
                        )
                        nc.tensor.matmul(
                            dps[:, lo:512],
                            ones8[:],
                            e2p[:, :, 0:w_],
                            start=(p == 0),
                            stop=(p == npair - 1),
                            perf_mode=Dd,
                        )
                        if filler:
                            filler.pop(0)()
                if c_hi < nch + 3:
                    return
                att_state.pop((h, b))
                recip = tp.tile([P, 512], BF16, name="recip")
                with nc.allow_low_precision(reason="bf16 softmax recip"):
                    nc.vector.reciprocal(recip[:], dps[:])
                tmp = tp.tile([P, 512], BF16, name="tmp")
                nc.vector.tensor_mul(tmp[:], yps[:], recip[:])
                nc.gpsimd.tensor_copy(y8[:, h, bs], tmp[:])
                nc.gpsimd.tensor_tensor(yr8[:, h, bs], tmp[:], y8[:, h, bs], SUB)

            def proj_tile(tt, nb):
                """One out tile: out[tt, nb] = sum_h y[:,h,tt].T @ wp (3-term)."""
                g = tt * NB + nb
                pool = vpsp if g % 3 == 2 else accp
                name = "vps" if g % 3 == 2 else "acc"
                ps3 = pool.tile([P, 512], F32, name=name)
                ts_ = slice(tt * P, (tt + 1) * P)
                ns = slice(nb * 512, (nb + 1) * 512)
                for hp in range(HL // 2):
                    hsl = slice(2 * hp, 2 * hp + 2)
                    nc.tensor.matmul(
                        ps3[:],
                        y8[:, hsl, ts_],
                        wp8[:, hsl, ns],
                        start=(hp == 0),
                        stop=False,
                        perf_mode=Dd,
                    )
                    nc.tensor.matmul(
                        ps3[:],
                        yr8[:, hsl, ts_],
                        wp8[:, hsl, ns],
                        start=False,
                        stop=False,
                        perf_mode=Dd,
                    )
                    nc.tensor.matmul(
                        ps3[:],
                        y8[:, hsl, ts_],
                        wpr8[:, hsl, ns],
                        start=False,
                        stop=(hp == HL // 2 - 1),
                        perf_mode=Dd,
                    )
                o_sb = op.tile([P, 512], BF16, name="o_sb")
                if g % 2 == 0:
                    nc.scalar.mul(o_sb[:], ps3[:], O_SCALE)
                else:
                    nc.vector.tensor_scalar_mul(o_sb[:], ps3[:], O_SCALE)
                nc.sync.dma_start(out_d[ts_, ns], o_sb[:])

            def proj_thunks(b):
                return [
                    (lambda tt=4 * b + t4, nb=nb: proj_tile(tt, nb))
                    for t4 in range(4)
                    for nb in range(NB)
                ]

            # --- fused pipeline: attention trails qkv by one piece; the
            # 16-chunk b=3 block is split across two pieces to level the
            # ACT exp load ---
            for h in range(HL):
                if h + 1 < HL:
                    load_w(h + 1)  # prefetch next head's weights
                if h == 2:
                    nc.sync.dma_start(
                        wp8[:], wp8_d.rearrange("(h p) n -> p h n", p=P)
                    )
                    nc.sync.dma_start(
                        wpr8[:], wpr8_d.rearrange("(h p) n -> p h n", p=P)
                    )
                for b in range(TB):
                    qkv_piece(h, b)
                    if h == 0:
                        if b >= 1:
                            att_block(0, b - 1)
                    elif b == 0:
                        att_block(h - 1, 3)
                    else:
                        att_block(h, b - 1)
            avail = []
            for b in range(TB - 1):
                avail.extend(proj_thunks(b))
            att_block(HL - 1, TB - 1, filler=avail)
            avail.extend(proj_thunks(TB - 1))
            for t in avail:
                t()

    nc.compile()
    return nc


def _rope_tables_T(T, head_dim):
    half = head_dim // 2
    inv_freq = 1.0 / (ROPE_THETA ** (np.arange(0, half, dtype=np.float64) / half))
    ang = np.arange(T, dtype=np.float64)[:, None] * inv_freq[None, :]  # [T, half]
    cos = np.concatenate([np.cos(ang), np.cos(ang)], axis=-1)  # [T, D]
    sin = np.concatenate([np.sin(ang), np.sin(ang)], axis=-1)
    return (
        np.ascontiguousarray(cos.T.astype(np.float32)),
        np.ascontiguousarray(sin.T.astype(np.float32)),
    )


_NC_CACHE = {}


def _get_nc(T, CIN, HL, COUT):
    key = (T, CIN, HL, COUT)
    if key not in _NC_CACHE:
        _NC_CACHE[key] = build_nc(T, CIN, HL, COUT)
    return _NC_CACHE[key]


def make_in_maps(x, w_attn, w_proj):
    import ml_dtypes

    f8 = ml_dtypes.float8_e4m3
    bf16 = ml_dtypes.bfloat16

    def q8pair(a, s):
        v8 = (np.asarray(a, np.float32) * s).astype(f8)
        r8 = (np.asarray(a, np.float32) * s - v8.astype(np.float32)).astype(f8)
        return v8, r8

    x = np.asarray(x)
    w_attn = np.asarray(w_attn)
    w_proj = np.asarray(w_proj)
    B, T, C = x.shape
    HL = NUM_HEADS // 2  # 8 heads per core
    CL = HL * D  # 1024
    CC = C // P

    cosT, sinT = _rope_tables_T(T, D)
    # fold SQK/(SX*SW) into the tables: psum is q*8192, out target q*16
    tab_scale = SQK / (SX * SW)
    cosT = (cosT * tab_scale).astype(bf16)
    sinT = (sinT * tab_scale).astype(bf16)

    # additive causal mask pattern: madd[p, 0, u] = -MBIG if u < p+384
    u = np.arange(896)[None, :]
    pp = np.arange(P)[:, None]
    madd = np.zeros((P, 2, 896), np.float32)
    madd[:, 0, :] = np.where(u < pp + 384, -MBIG, 0.0)
    madd = madd.astype(f8)
    id2 = np.zeros((P, 2, P), np.float32)
    id2[:, 0, :] = np.eye(P) * MBIG
    id2 = id2.astype(f8)

    wp_shards = []
    w_shards = []
    for g in range(2):
        qkv_cols = [
            w_attn[:, g * CL : (g + 1) * CL],
            w_attn[:, C + g * CL : C + (g + 1) * CL],
            w_attn[:, 2 * C + g * CL : 2 * C + (g + 1) * CL],
        ]
        w_shard = np.empty((HL, 3, P, CC * D), dtype=np.float32)
        for j, wj in enumerate(qkv_cols):
            s = wj.reshape(CC, P, HL, D).transpose(2, 1, 0, 3)  # [HL, P, CC, D]
            w_shard[:, j] = s.reshape(HL, P, CC * D)
        w_shards.append(q8pair(w_shard, SW))
        wp_shards.append(
            q8pair(np.ascontiguousarray(w_proj[g * CL : (g + 1) * CL, :]), SWP)
        )

    in_maps = []
    for s in range(8):
        b, g = s // 2, s % 2
        xT = np.ascontiguousarray(x[b].T)
        x8, xr8 = q8pair(xT, SX)
        x2 = np.ascontiguousarray(np.stack([x8, xr8], axis=0)).reshape(2 * C, T)
        w8, wr8 = w_shards[g]
        wp8, wpr8 = wp_shards[g]
        in_maps.append(
            {
                "x2": x2,
                "w8": w8,
                "wr8": wr8,
                "wp8": wp8,
                "wpr8": wpr8,
                "cosT": cosT,
                "sinT": sinT,
                "madd": madd,
                "id2": id2,
            }
        )
    return in_maps


def combine(results, x_shape):
    B, T, C = x_shape
    out = np.empty((B, T, C), dtype=np.float32)
    for b in range(B):
        out[b] = results[2 * b]["out"].astype(np.float32) + results[
            2 * b + 1
        ]["out"].astype(np.float32)
    return out


def kernel(x, w_attn, w_proj):
    from concourse.bass_utils import run_bass_kernel_spmd

    x = np.asarray(x)
    B, T, C = x.shape  # 4, 2048, 2048
    HL = NUM_HEADS // 2

    nc = _get_nc(T, C, HL, C)
    in_maps = make_in_maps(x, w_attn, w_proj)
    res = run_bass_kernel_spmd(nc, in_maps, list(range(8)))
    return combine(res.results, (B, T, C))


# revision 37
# speedup vs baseline: 1.0071x; 1.0053x over previous
"""Causal self-attention (RoPE, 16 heads) on 8 Trainium2 NeuronCores — v3 fp8.

Sharding: core s -> (batch b = s//2, head-half g = s%2). Each core computes
qkv = x_b @ w_attn[:, heads g], RoPE, causal SDPA for its 8 heads, and a
partial y_local @ w_proj[rows g] -> [T, C]. Host sums the two partials per
batch (row-parallel Megatron unshard).

v3: fp8e4m3 DoubleRow matmuls with error compensation.
 - DoubleRow fp8 matmuls process two K-tiles per instruction at 0.5
   cycles/row (2x bf16 FLOPs, 4x when both slots carry fresh data).
 - qkv + proj gemms: 3-term compensated products over chunk pairs
   (a8@b8 + ar8@b8 + a8@br8 with (value, residual) fp8 pairs) = bf16-level
   accuracy at 0.75x the bf16 cycle cost. x/w/wp residuals from host.
 - scores: two DR matmuls per chunk: (k8,kr8) x q8-dup + (k8,kr8) x
   qr8-dup = exact (k8+kr8)(q8+qr8), so no score-path noise rides.
 - attention weights e8 = exp(s) in fp8 (noise ~1.4e-2, partially
   cancelled by the shared-denominator normalization).
 - causal mask: additive -57600 matmuls accumulated straight into the
   score PSUM group (identity x mask-pattern), no vector masking.
 - softmax denominator: ones-DR matmul accumulation over e8 pairs
   (column-sum broadcast to all partitions), recip on DVE.
 - RoPE: rotate on Pool (partition-shifted copies), combine muls split
   DVE/Pool, raw PSUM->SBUF copies on ACT.
"""

import sys

sys.path.insert(0, "/opt/trn_rl_repo")

import numpy as np

import concourse.bacc as bacc
import concourse.mybir as mybir
import concourse.tile as tile

P = 128
D = 128
F32 = mybir.dt.float32
BF16 = mybir.dt.bfloat16
FP8 = mybir.dt.float8e4
EXP = mybir.ActivationFunctionType.Exp
Dd = mybir.MatmulPerfMode.DoubleRow
MUL = mybir.AluOpType.mult
SUB = mybir.AluOpType.subtract

NUM_HEADS = 16
ROPE_THETA = 10000.0

# fp8 scales
SX = 16.0      # x8 = x * 16
SW = 512.0     # w8 = w * 512
SQK = 16.0     # q8/k8 post-rope
SV = 8.0       # v8
SY = 8.0       # y8
SWP = 512.0    # wp8
MBIG = 240.0   # mask magnitude (id2 * madd -> -57600 in score psum)


def build_nc(
    T=2048,
    CIN=2048,
    HL=8,
    COUT=2048,
    *,
    w_bufs=2,
    acc_bufs=2,
    st_bufs=3,
    e_bufs=3,
    qk_bufs=2,
    v_bufs=2,
    o_bufs=4,
    rope_bufs=2,
    tmp_bufs=2,
):
    CC = CIN // P        # contraction chunks (16)
    CP = CC // 2         # contraction chunk pairs (8)
    TB = T // 512        # 512-wide t blocks (4)
    TC = T // P          # 128-wide t chunks (16)
    NB = COUT // 512     # output col blocks (4)
    SCALE = 1.0 / float(np.sqrt(D))
    EXP_SCALE = SCALE / (SQK * SQK)     # score psum is q8*k8 = s*256
    V_SCALE = SV / (SX * SW)            # v psum is x8*w8 = v*8192
    O_SCALE = 1.0 / (SY * SWP)          # proj psum is y8*wp8 = out*4096

    nc = bacc.Bacc("TRN2", target_bir_lowering=False, debug=False)

    x2_d = nc.dram_tensor("x2", [2 * CIN, T], FP8, kind="ExternalInput").ap()
    w8_d = nc.dram_tensor("w8", [HL, 3, P, CC * D], FP8, kind="ExternalInput").ap()
    wr8_d = nc.dram_tensor("wr8", [HL, 3, P, CC * D], FP8, kind="ExternalInput").ap()
    wp8_d = nc.dram_tensor("wp8", [HL * D, COUT], FP8, kind="ExternalInput").ap()
    wpr8_d = nc.dram_tensor("wpr8", [HL * D, COUT], FP8, kind="ExternalInput").ap()
    cosT_d = nc.dram_tensor("cosT", [D, T], BF16, kind="ExternalInput").ap()
    sinT_d = nc.dram_tensor("sinT", [D, T], BF16, kind="ExternalInput").ap()
    madd_d = nc.dram_tensor("madd", [P, 2, 896], FP8, kind="ExternalInput").ap()
    id2_d = nc.dram_tensor("id2", [P, 2, P], FP8, kind="ExternalInput").ap()
    out_d = nc.dram_tensor("out", [T, COUT], BF16, kind="ExternalOutput").ap()

    with tile.TileContext(nc) as tc:
        with (
            tc.tile_pool(name="const", bufs=1) as cp,
            tc.tile_pool(name="xt", bufs=1) as xtp,
            tc.tile_pool(name="ropetab", bufs=1) as rtp,
            tc.tile_pool(name="maskp", bufs=1) as mp,
            tc.tile_pool(name="yt", bufs=1) as ytp,
            tc.tile_pool(name="wpp", bufs=1) as wpp,
            tc.tile_pool(name="w1", bufs=w_bufs) as wpool,
            tc.tile_pool(name="rope", bufs=rope_bufs) as rp,
            tc.tile_pool(name="qk", bufs=qk_bufs) as qkp,
            tc.tile_pool(name="vpool", bufs=v_bufs) as vp,
            tc.tile_pool(name="esb", bufs=e_bufs) as ep,
            tc.tile_pool(name="tmpp", bufs=tmp_bufs) as tp,
            tc.tile_pool(name="o", bufs=o_bufs) as op,
            tc.tile_pool(name="ps_acc", bufs=acc_bufs, space="PSUM") as accp,
            tc.tile_pool(name="ps_v", bufs=1, space="PSUM") as vpsp,
            tc.tile_pool(name="ps_st", bufs=st_bufs, space="PSUM") as stp,
            tc.tile_pool(name="ps_d", bufs=1, space="PSUM") as dpsp,
            tc.tile_pool(name="ps_y", bufs=1, space="PSUM") as ypsp,
        ):
            # constants
            ones8 = cp.tile([P, 2, P], FP8)
            nc.vector.memset(ones8[:], 1.0)

            # resident tensors (x8 and residual xr8 share one tile/stream)
            x2 = xtp.tile([P, 2, CC, T], FP8)
            x8 = x2[:, 0]
            xr8 = x2[:, 1]
            cosT = rtp.tile([D, T], BF16)
            sinT = rtp.tile([D, T], BF16)
            madd = mp.tile([P, 2, 896], FP8)
            id2 = mp.tile([P, 2, P], FP8)
            y8 = ytp.tile([P, HL, T], FP8)
            yr8 = ytp.tile([P, HL, T], FP8)
            wp8 = wpp.tile([P, HL, COUT], FP8)
            wpr8 = wpp.tile([P, HL, COUT], FP8)

            # --- startup DMAs (SP queue order matters) ---
            x2r = x2_d.rearrange("(s c p) t -> p s c t", p=P, s=2)
            w_tiles = {}

            def load_w(h, interleave=False):
                w8f = wpool.tile([P, 3 * CC * D], FP8, name="w8_h")
                wr8f = wpool.tile([P, 3 * CC * D], FP8, name="wr8_h")
                w8h = w8f.rearrange("p (j c d) -> p j c d", j=3, c=CC)
                wr8h = wr8f.rearrange("p (j c d) -> p j c d", j=3, c=CC)
                wv8 = w8_d[h].rearrange("j p k -> p j k")
                wvr8 = wr8_d[h].rearrange("j p k -> p j k")
                if interleave:
                    return (w8h, wr8h, w8f, wr8f, wv8, wvr8)
                nc.sync.dma_start(
                    w8f.rearrange("p (j k) -> p j k", j=3)[:], wv8
                )
                nc.sync.dma_start(
                    wr8f.rearrange("p (j k) -> p j k", j=3)[:], wvr8
                )
                w_tiles[h] = (w8h, wr8h)

            w8h0, wr8h0, w8f0, wr8f0, wv80, wvr80 = load_w(0, interleave=True)
            w_tiles[0] = (w8h0, wr8h0)
            KD = CC * D
            nc.sync.dma_start(x2[:, :, :, 0:128], x2r[:, :, :, 0:128])
            nc.sync.dma_start(w8f0[:, 0:KD], wv80[:, 0])
            nc.sync.dma_start(wr8f0[:, 0:KD], wvr80[:, 0])
            nc.sync.dma_start(x2[:, :, :, 128:256], x2r[:, :, :, 128:256])
            nc.sync.dma_start(w8f0[:, KD : 2 * KD], wv80[:, 1])
            nc.sync.dma_start(wr8f0[:, KD : 2 * KD], wvr80[:, 1])
            nc.sync.dma_start(cosT[:], cosT_d[:])
            nc.sync.dma_start(sinT[:], sinT_d[:])
            nc.sync.dma_start(w8f0[:, 2 * KD :], wv80[:, 2])
            nc.sync.dma_start(wr8f0[:, 2 * KD :], wvr80[:, 2])
            nc.sync.dma_start(x2[:, :, :, 256:512], x2r[:, :, :, 256:512])
            nc.sync.dma_start(madd[:], madd_d[:])
            nc.sync.dma_start(id2[:], id2_d[:])
            for tb in range(1, TB):
                s = slice(tb * 512, (tb + 1) * 512)
                nc.sync.dma_start(x2[:, :, :, s], x2r[:, :, :, s])

            qk_tiles = {}
            v_tiles = {}

            def rope_combine(j, acc, h, b, lo, hi):
                """acc (psum, q*8192) -> q8 / (k8, kr8) fp8 at scale 16."""
                w_ = hi - lo
                hs = slice(b * 512 + lo, b * 512 + hi)
                q8t, k82 = qk_tiles[h]
                raw = rp.tile([P, 512], BF16, name="raw")
                # b==0 pieces run beside a 16-chunk att block: ACT is
                # saturated with exps there, so spill the raw copy to DVE
                if b == 0:
                    nc.vector.tensor_copy(raw[:, 0:w_], acc[:, 0:w_])
                else:
                    nc.scalar.copy(raw[:, 0:w_], acc[:, 0:w_])
                rot = rp.tile([P, 512], BF16, name="rot")
                nc.vector.tensor_scalar_mul(
                    rot[0:64, 0:w_], raw[64:128, 0:w_], -1.0
                )
                nc.vector.tensor_copy(rot[64:128, 0:w_], raw[0:64, 0:w_])
                qc = rp.tile([P, 512], BF16, name="qc")
                # cos/sin tables pre-scaled by SQK/(SX*SW) on host
                nc.vector.tensor_mul(qc[:, 0:w_], raw[:, 0:w_], cosT[:, hs])
                qs = rp.tile([P, 512], BF16, name="qs")
                nc.gpsimd.tensor_mul(qs[:, 0:w_], rot[:, 0:w_], sinT[:, hs])
                dst = q8t if j == 0 else k82
                kbf = rp.tile([P, 512], BF16, name="kbf")
                nc.vector.tensor_add(kbf[:, 0:w_], qc[:, 0:w_], qs[:, 0:w_])
                nc.gpsimd.tensor_copy(dst[:, 0, hs], kbf[:, 0:w_])
                nc.vector.tensor_tensor(
                    dst[:, 1, hs], kbf[:, 0:w_], dst[:, 0, hs], SUB
                )

            def qkv_piece(h, b):
                """q8/k82 for t-block b (with RoPE) + v8/vr8 t-chunks 4b..4b+3."""
                w8h, wr8h = w_tiles[h]
                if b == 0:
                    q8t = qkp.tile([P, 2, T], FP8, name="q8t")
                    k82 = qkp.tile([P, 2, T], FP8, name="k82")
                    v8 = vp.tile([P, TC, D], FP8, name="v8")
                    vr8 = vp.tile([P, TC, D], FP8, name="vr8")
                    qk_tiles[h] = (q8t, k82)
                    v_tiles[h] = (v8, vr8)
                v8, vr8 = v_tiles[h]
                halves = (
                    [(0, 128), (128, 256), (256, 512)]
                    if (h == 0 and b == 0)
                    else [(0, 512)]
                )
                for lo, hi in halves:
                    w_ = hi - lo
                    hs = slice(b * 512 + lo, b * 512 + hi)
                    for j in range(2):  # q, k gemms: 3-term over chunk pairs
                        acc = accp.tile([P, 512], F32, name="acc")
                        for cpi in range(CP):
                            c = 2 * cpi
                            cs = slice(c, c + 2)
                            nc.tensor.matmul(
                                acc[:, 0:w_],
                                w8h[:, j, cs],
                                x8[:, cs, hs],
                                start=(cpi == 0),
                                stop=False,
                                perf_mode=Dd,
                            )
                            nc.tensor.matmul(
                                acc[:, 0:w_],
                                wr8h[:, j, cs],
                                x8[:, cs, hs],
                                start=False,
                                stop=False,
                                perf_mode=Dd,
                            )
                            nc.tensor.matmul(
                                acc[:, 0:w_],
                                w8h[:, j, cs],
                                xr8[:, cs, hs],
                                start=False,
                                stop=(cpi == CP - 1),
                                perf_mode=Dd,
                            )
                        rope_combine(j, acc, h, b, lo, hi)
                    # V in [t, d] layout: x-stationary 3-term
                    vps = vpsp.tile([P, 4, P], F32, name="vps")
                    t4s = range(lo // P, hi // P)
                    for t4 in t4s:
                        tt = 4 * b + t4
                        ts_ = slice(tt * P, (tt + 1) * P)
                        for cpi in range(CP):
                            cs = slice(2 * cpi, 2 * cpi + 2)
                            nc.tensor.matmul(
                                vps[:, t4],
                                x8[:, cs, ts_],
                                w8h[:, 2, cs],
                                start=(cpi == 0),
                                stop=False,
                                perf_mode=Dd,
                            )
                            nc.tensor.matmul(
                                vps[:, t4],
                                xr8[:, cs, ts_],
                                w8h[:, 2, cs],
                                start=False,
                                stop=False,
                                perf_mode=Dd,
                            )
                            nc.tensor.matmul(
                                vps[:, t4],
                                x8[:, cs, ts_],
                                wr8h[:, 2, cs],
                                start=False,
                                stop=(cpi == CP - 1),
                                perf_mode=Dd,
                            )
                    vsl = slice(4 * b + t4s.start, 4 * b + t4s.stop)
                    if b == 0:
                        nc.vector.tensor_scalar_mul(
                            v8[:, vsl, :], vps[:, t4s.start : t4s.stop], V_SCALE
                        )
                    else:
                        nc.scalar.mul(
                            v8[:, vsl, :], vps[:, t4s.start : t4s.stop], V_SCALE
                        )
                    nc.vector.scalar_tensor_tensor(
                        vr8[:, vsl, :],
                        vps[:, t4s.start : t4s.stop],
                        V_SCALE,
                        v8[:, vsl, :],
                        MUL,
                        SUB,
                    )

            att_state = {}

            def att_block(h, b, filler=None, c_lo=0, c_hi=None):
                """Causal attention for head h, q block b -> y8/yr8[:, h, block].

                Steps [c_lo, c_hi) of the chunk loop; a big block can be
                split across two qkv pieces to level the ACT exp load.
                """
                q8t, k82 = qk_tiles[h]
                v8, vr8 = v_tiles[h]
                nch = 4 * (b + 1)
                npair = nch // 2
                bs = slice(b * 512, (b + 1) * 512)
                if c_hi is None:
                    c_hi = nch + 3
                if c_lo == 0:
                    yps = ypsp.tile([P, 512], F32, name="yps")
                    dps = dpsp.tile([P, 512], F32, name="dps")
                    es = {}
                    att_state[(h, b)] = (yps, dps, es)
                else:
                    yps, dps, es = att_state[(h, b)]
                qlo = {nch - 2: 256, nch - 1: 256}
                # software-pipelined: y(pair p) trails scores by 2 pairs so
                # the ACT exp chain is never on the PE critical path
                for c in range(c_lo, c_hi):
                    if c < nch:
                        lo = qlo.get(c, 0)
                        w_ = 512 - lo
                        st = stp.tile([P, 512], F32, name="st")
                        j = c - (nch - 4)
                        ks_ = k82[:, :, c * P : (c + 1) * P]
                        # cols [lo, 128*j) are fully below the causal
                        # diagonal: fill them with the constant mask-add
                        # alone, no scores needed there
                        slo = max(lo, 128 * j) if j > 0 else lo
                        if slo > lo:
                            ms0 = 384 - 128 * j + lo
                            nc.tensor.matmul(
                                st[:, 0 : slo - lo],
                                id2[:],
                                madd[:, :, ms0 : ms0 + slo - lo],
                                start=True,
                                stop=True,
                                perf_mode=Dd,
                            )
                        sw = 512 - slo
                        soff = slo - lo
                        qs_ = slice(b * 512 + slo, (b + 1) * 512)
                        nc.tensor.matmul(
                            st[:, soff : soff + sw],
                            ks_,
                            q8t[:, 0:1, qs_].broadcast_to((P, 2, sw)),
                            start=True,
                            stop=False,
                            perf_mode=Dd,
                        )
                        nc.tensor.matmul(
                            st[:, soff : soff + sw],
                            ks_,
                            q8t[:, 1:2, qs_].broadcast_to((P, 2, sw)),
                            start=False,
                            stop=(j < 0),
                            perf_mode=Dd,
                        )
                        if j >= 0:
                            # additive causal mask into the same psum group
                            wj = 128 * (j + 1) - slo
                            ms = 384 - 128 * j + slo
                            nc.tensor.matmul(
                                st[:, soff : soff + wj],
                                id2[:],
                                madd[:, :, ms : ms + wj],
                                start=False,
                                stop=True,
                                perf_mode=Dd,
                            )
                        if c % 2 == 0:
                            e2 = ep.tile([P, 2, 512], FP8, name="e2")
                            es[c // 2] = e2
                        e2 = es[c // 2]
                        nc.scalar.activation(
                            e2[:, c % 2, 0:w_], st[:, 0:w_], EXP, scale=EXP_SCALE
                        )
                    if c >= 4 and c % 2 == 0:
                        p = c // 2 - 2
                        lo = qlo.get(2 * p, 0)
                        w_ = 512 - lo
                        e2p = es.pop(p)
                        vs = slice(2 * p, 2 * p + 2)
                        nc.tensor.matmul(
                            yps[:, lo:512],
                            v8[:, vs, :],
                            e2p[:, :, 0:w_],
                            start=(p == 0),
                            stop=False,
                            perf_mode=Dd,
                        )
                        nc.tensor.matmul(
                            yps[:, lo:512],
                            vr8[:, vs, :],
                            e2p[:, :, 0:w_],
                            start=False,
                            stop=(p == npair - 1),
                            perf_mode=Dd,
                        )
                        nc.tensor.matmul(
                            dps[:, lo:512],
                            ones8[:],
                            e2p[:, :, 0:w_],
                            start=(p == 0),
                            stop=(p == npair - 1),
                            perf_mode=Dd,
                        )
                        if filler:
                            filler.pop(0)()
                if c_hi < nch + 3:
                    return
                att_state.pop((h, b))
                recip = tp.tile([P, 512], BF16, name="recip")
                with nc.allow_low_precision(reason="bf16 softmax recip"):
                    nc.vector.reciprocal(recip[:], dps[:])
                tmp = tp.tile([P, 512], BF16, name="tmp")
                nc.vector.tensor_mul(tmp[:], yps[:], recip[:])
                nc.gpsimd.tensor_copy(y8[:, h, bs], tmp[:])
                nc.gpsimd.tensor_tensor(yr8[:, h, bs], tmp[:], y8[:, h, bs], SUB)

            def proj_tile(tt, nb):
                """One out tile: out[tt, nb] = sum_h y[:,h,tt].T @ wp (3-term)."""
                g = tt * NB + nb
                pool = vpsp if g % 3 == 2 else accp
                name = "vps" if g % 3 == 2 else "acc"
                ps3 = pool.tile([P, 512], F32, name=name)
                ts_ = slice(tt * P, (tt + 1) * P)
                ns = slice(nb * 512, (nb + 1) * 512)
                for hp in range(HL // 2):
                    hsl = slice(2 * hp, 2 * hp + 2)
                    nc.tensor.matmul(
                        ps3[:],
                        y8[:, hsl, ts_],
                        wp8[:, hsl, ns],
                        start=(hp == 0),
                        stop=False,
                        perf_mode=Dd,
                    )
                    nc.tensor.matmul(
                        ps3[:],
                        yr8[:, hsl, ts_],
                        wp8[:, hsl, ns],
                        start=False,
                        stop=False,
                        perf_mode=Dd,
                    )
                    nc.tensor.matmul(
                        ps3[:],
                        y8[:, hsl, ts_],
                        wpr8[:, hsl, ns],
                        start=False,
                        stop=(hp == HL // 2 - 1),
                        perf_mode=Dd,
                    )
                o_sb = op.tile([P, 512], BF16, name="o_sb")
                if g % 2 == 0:
                    nc.scalar.mul(o_sb[:], ps3[:], O_SCALE)
                else:
                    nc.vector.tensor_scalar_mul(o_sb[:], ps3[:], O_SCALE)
                nc.sync.dma_start(out_d[ts_, ns], o_sb[:])

            def proj_thunks(b):
                return [
                    (lambda tt=4 * b + t4, nb=nb: proj_tile(tt, nb))
                    for t4 in range(4)
                    for nb in range(NB)
                ]

            # --- fused pipeline: attention trails qkv by one piece; the
            # 16-chunk b=3 block is split across two pieces to level the
            # ACT exp load ---
            for h in range(HL):
                if h + 1 < HL:
                    load_w(h + 1)  # prefetch next head's weights
                if h == 2:
                    nc.sync.dma_start(
                        wp8[:], wp8_d.rearrange("(h p) n -> p h n", p=P)
                    )
                    nc.sync.dma_start(
                        wpr8[:], wpr8_d.rearrange("(h p) n -> p h n", p=P)
                    )
                for b in range(TB):
                    qkv_piece(h, b)
                    if h == 0:
                        if b >= 1:
                            att_block(0, b - 1)
                    elif b == 0:
                        att_block(h - 1, 3)
                    else:
                        att_block(h, b - 1)
            avail = []
            for b in range(TB - 1):
                avail.extend(proj_thunks(b))
            att_block(HL - 1, TB - 1, filler=avail)
            avail.extend(proj_thunks(TB - 1))
            for t in avail:
                t()

    nc.compile()
    return nc


def _rope_tables_T(T, head_dim):
    half = head_dim // 2
    inv_freq = 1.0 / (ROPE_THETA ** (np.arange(0, half, dtype=np.float64) / half))
    ang = np.arange(T, dtype=np.float64)[:, None] * inv_freq[None, :]  # [T, half]
    cos = np.concatenate([np.cos(ang), np.cos(ang)], axis=-1)  # [T, D]
    sin = np.concatenate([np.sin(ang), np.sin(ang)], axis=-1)
    return (
        np.ascontiguousarray(cos.T.astype(np.float32)),
        np.ascontiguousarray(sin.T.astype(np.float32)),
    )


_NC_CACHE = {}


def _get_nc(T, CIN, HL, COUT):
    key = (T, CIN, HL, COUT)
    if key not in _NC_CACHE:
        _NC_CACHE[key] = build_nc(T, CIN, HL, COUT)
    return _NC_CACHE[key]


def make_in_maps(x, w_attn, w_proj):
    import ml_dtypes

    f8 = ml_dtypes.float8_e4m3
    bf16 = ml_dtypes.bfloat16

    def q8pair(a, s):
        v8 = (np.asarray(a, np.float32) * s).astype(f8)
        r8 = (np.asarray(a, np.float32) * s - v8.astype(np.float32)).astype(f8)
        return v8, r8

    x = np.asarray(x)
    w_attn = np.asarray(w_attn)
    w_proj = np.asarray(w_proj)
    B, T, C = x.shape
    HL = NUM_HEADS // 2  # 8 heads per core
    CL = HL * D  # 1024
    CC = C // P

    cosT, sinT = _rope_tables_T(T, D)
    # fold SQK/(SX*SW) into the tables: psum is q*8192, out target q*16
    tab_scale = SQK / (SX * SW)
    cosT = (cosT * tab_scale).astype(bf16)
    sinT = (sinT * tab_scale).astype(bf16)

    # additive causal mask pattern: madd[p, 0, u] = -MBIG if u < p+384
    u = np.arange(896)[None, :]
    pp = np.arange(P)[:, None]
    madd = np.zeros((P, 2, 896), np.float32)
    madd[:, 0, :] = np.where(u < pp + 384, -MBIG, 0.0)
    madd = madd.astype(f8)
    id2 = np.zeros((P, 2, P), np.float32)
    id2[:, 0, :] = np.eye(P) * MBIG
    id2 = id2.astype(f8)

    wp_shards = []
    w_shards = []
    for g in range(2):
        qkv_cols = [
            w_attn[:, g * CL : (g + 1) * CL],
            w_attn[:, C + g * CL : C + (g + 1) * CL],
            w_attn[:, 2 * C + g * CL : 2 * C + (g + 1) * CL],
        ]
        w_shard = np.empty((HL, 3, P, CC * D), dtype=np.float32)
        for j, wj in enumerate(qkv_cols):
            s = wj.reshape(CC, P, HL, D).transpose(2, 1, 0, 3)  # [HL, P, CC, D]
            w_shard[:, j] = s.reshape(HL, P, CC * D)
        w_shards.append(q8pair(w_shard, SW))
        wp_shards.append(
            q8pair(np.ascontiguousarray(w_proj[g * CL : (g + 1) * CL, :]), SWP)
        )

    in_maps = []
    for s in range(8):
        b, g = s // 2, s % 2
        xT = np.ascontiguousarray(x[b].T)
        x8, xr8 = q8pair(xT, SX)
        x2 = np.ascontiguousarray(np.stack([x8, xr8], axis=0)).reshape(2 * C, T)
        w8, wr8 = w_shards[g]
        wp8, wpr8 = wp_shards[g]
        in_maps.append(
            {
                "x2": x2,
                "w8": w8,
                "wr8": wr8,
                "wp8": wp8,
                "wpr8": wpr8,
                "cosT": cosT,
                "sinT": sinT,
                "madd": madd,
                "id2": id2,
            }
        )
    return in_maps


def combine(results, x_shape):
    B, T, C = x_shape
    out = np.empty((B, T, C), dtype=np.float32)
    for b in range(B):
        out[b] = results[2 * b]["out"].astype(np.float32) + results[
            2 * b + 1
        ]["out"].astype(np.float32)
    return out


def kernel(x, w_attn, w_proj):
    from concourse.bass_utils import run_bass_kernel_spmd

    x = np.asarray(x)
    B, T, C = x.shape  # 4, 2048, 2048
    HL = NUM_HEADS // 2

    nc = _get_nc(T, C, HL, C)
    in_maps = make_in_maps(x, w_attn, w_proj)
    res = run_bass_kernel_spmd(nc, in_maps, list(range(8)))
    return combine(res.results, (B, T, C))
